# revision 21
# baseline (speedup 1.0000x reference)
"""Trainium2 Bass kernel for a 2-layer Mistral-style VLM block (tensor-parallel, 8 cores).

v3 strategy (on top of v2's fp16 GEMMs / folded LoRA+ln / batch-pipelined AR):
- The AllReduce carries the residual stream: each core adds h/8 to its partial
  (Sum_r(partial_r + h/8) = h + Sum partial), so the AR output IS the new h.
  Kills all per-site h DRAM round-trips (load h + ar, add, write back).
- xmega (the resident activation megatile) is double-buffered so batch b1's
  fill + rmsnorm overlaps batch b0's GEMMs.
- Each AR is split into 2 chunks along k-tiles so the collective starts while
  the producer GEMMs still run and the consumer fill starts on chunk 0 while
  chunk 1 is in flight.
- q/k head tiles stay in SBUF (no DRAM bounce).
- Output written fp16 (host converts to f32), halving the output-write tail.
- Wo/Wd PSUM evacuation moved to DVE scalar_tensor_tensor (adds h/8 in the
  same op), offloading ScalarE which is busy with exp during attention.
"""

import os
import sys

sys.path.insert(0, '/opt/trn_rl_repo')

import numpy as np
import ml_dtypes

NCORES = 8
D, VH, DFF, NL, VOCAB, NH, NKV, HD, RK, SCALE = 4096, 1024, 14336, 2, 32000, 32, 8, 128, 8, 4.0
B, NIMG, T = 2, 257, 511
S = NIMG + T            # 768
NTOK = B * S            # 1536
DSH = D // NCORES       # 512
FSH = DFF // NCORES     # 1792
KT = D // 128           # 32
FT = FSH // 128         # 14
VK = VH // 128          # 8
QH = NH // NCORES       # 4
CH = 384
NCH = S // CH           # 2
EPS = 1e-5
ISQ = 1.0 / float(np.sqrt(HD))
EXP_BIAS = 0.0   # exp(s) <= e^10.8 < 65504 fits fp16; rowsum stays in normal range
MASK_NEG = -1e30
NIMGP = NIMG + 1          # pad to even free size
KC = KT // 2              # 16 k-tiles per AR chunk

BF16 = ml_dtypes.bfloat16
F16NP = np.float16
_PROGRAM = None


def _bf(x):
    return np.ascontiguousarray(np.asarray(x, np.float32).astype(BF16))


def _h(x):
    return np.ascontiguousarray(np.asarray(x, np.float32).astype(F16NP))


def _build_program():
    import concourse.bass as bass
    import concourse.bacc as bacc
    import concourse.mybir as mybir
    import concourse.tile as tile

    F32 = mybir.dt.float32
    F32R = mybir.dt.float32r
    F16 = mybir.dt.float16
    BF = mybir.dt.bfloat16
    AF = mybir.ActivationFunctionType
    ALU = mybir.AluOpType
    AF_SILU = AF.Sigmoid if os.environ.get('KSIM') == '1' else AF.Silu

    nc = bacc.Bacc("TRN2", target_bir_lowering=False)

    img_in = nc.dram_tensor("img", [128, VK * B * NIMGP], F16, kind="ExternalInput")
    projw_in = nc.dram_tensor("projw", [KT // 4, 128, 4 * VK * 128], F16, kind="ExternalInput")
    projb_in = nc.dram_tensor("projb", [128, KT], F32, kind="ExternalInput")
    txt_in = nc.dram_tensor("txt", [128, KT, B * T], F16, kind="ExternalInput")
    cos_in = nc.dram_tensor("cos_t", [128, S], F16, kind="ExternalInput")
    sin_in = nc.dram_tensor("sin_t", [128, S], F16, kind="ExternalInput")   # sign-folded
    mask_in = nc.dram_tensor("mbig", [128, 640], BF, kind="ExternalInput")
    onesb_in = nc.dram_tensor("onesb", [128, 1], F16, kind="ExternalInput")
    onesrow_in = nc.dram_tensor("onesrow", [1, 128], F16, kind="ExternalInput")
    onesrow32_in = nc.dram_tensor("onesrow32", [1, 128], F32, kind="ExternalInput")
    sel4_in = nc.dram_tensor("sel4", [128, 1], mybir.dt.float32r, kind="ExternalInput")
    ident_in = nc.dram_tensor("ident", [128, 128], F16, kind="ExternalInput")
    lnf_in = nc.dram_tensor("lnf", [128, KT], F32, kind="ExternalInput")
    wqkv_in = [nc.dram_tensor(f"wqkv{l}", [6, 128, KT * 128], F16, kind="ExternalInput") for l in range(NL)]
    wo_in = [nc.dram_tensor(f"wo{l}", [KT // 4, 128, 4 * QH * 128], F16, kind="ExternalInput") for l in range(NL)]
    wgu_in = [nc.dram_tensor(f"wgu{l}", [FT, 2, 128, KT * 128], F16, kind="ExternalInput") for l in range(NL)]
    wd_in = [nc.dram_tensor(f"wd{l}", [KT // 2, 128, 2 * FT * 128], F16, kind="ExternalInput") for l in range(NL)]
    out_ext = nc.dram_tensor("out", [128, KT, NTOK], F16, kind="ExternalOutput")

    RG = [list(range(NCORES))]

    with tile.TileContext(nc) as tc:
        with tc.tile_pool(name="sb", bufs=1) as sb, \
             tc.tile_pool(name="ps", bufs=1, space="PSUM") as ps, \
             tc.tile_pool(name="dram", bufs=1, space="DRAM") as dram:

            # ---- resident constants ----
            cos_sb = sb.tile([128, S], F16, tag="res_cos", bufs=1)
            sin_sb = sb.tile([128, S], F16, tag="res_sin", bufs=1)
            onesb_sb = sb.tile([128, 1], F16, tag="res_onesb", bufs=1)
            onesrow_sb = sb.tile([1, 128], F16, tag="res_onesrow", bufs=1)
            onesrow32_sb = sb.tile([1, 128], F32, tag="res_onesrow32", bufs=1)
            ident_sb = sb.tile([128, 128], F16, tag="res_ident", bufs=1)
            projb_sb = sb.tile([128, KT], F32, tag="res_projb", bufs=1)
            lnf_sb = sb.tile([128, KT], F32, tag="res_lnf", bufs=1)
            for t_, i_ in [(cos_sb, cos_in), (sin_sb, sin_in), (onesb_sb, onesb_in),
                           (onesrow_sb, onesrow_in), (onesrow32_sb, onesrow32_in),
                           (ident_sb, ident_in),
                           (projb_sb, projb_in), (lnf_sb, lnf_in)]:
                nc.sync.dma_start(t_[:], i_[:])
            mask_sb = sb.tile([128, 640], BF, tag="res_mask", bufs=1, name="mskbig")
            nc.sync.dma_start(mask_sb[:], mask_in[:])
            sel4_sb = sb.tile([128, 1], F32R, tag="res_sel4", bufs=1)
            nc.sync.dma_start(sel4_sb[:], sel4_in[:])
            eps_sb = sb.tile([128, 1], F32, tag="res_eps", bufs=1)
            nb_sb = sb.tile([128, 1], F32, tag="res_nb", bufs=1)
            nc.vector.memset(eps_sb[:], EPS)
            nc.vector.memset(nb_sb[:], EXP_BIAS)

            # phase-0 output: initial h per batch (img tokens from projector,
            # txt tokens DMA'd straight from host-prepped embedding rows)
            h_init = [dram.tile([128, KT, S], F16, tag=f"hinit{b}", bufs=1, name=f"h_i{b}")
                      for b in range(B)]

            # ---- phase 0: replicated projector (no collective); txt DMA on the
            # scalar HWDGE ring so it does not block img/weight loads on sync ----
            img_sb = sb.tile([128, VK * B * NIMGP], F16, tag="imgt", bufs=1, name="img_sb")
            nc.sync.dma_start(img_sb[:], img_in[:])
            for b in range(B):
                nc.scalar.dma_start(h_init[b][:, :, NIMG:S], txt_in[:, :, b * T:(b + 1) * T])
            for g in range(KT // 4):
                pw = sb.tile([128, 4 * VK * 128], F16, tag="wbig", bufs=2, name=f"pw{g}")
                nc.sync.dma_start(pw[:], projw_in[g])
                for mm in range(4):
                    m = 4 * g + mm
                    for b in range(B):
                        pt = ps.tile([128, NIMGP], F32, tag="ps1", bufs=3, name=f"pj{g}{mm}{b}")
                        for k in range(VK):
                            nc.tensor.matmul(pt[:], pw[:, (mm * VK + k) * 128:(mm * VK + k + 1) * 128],
                                             img_sb[:, (k * B + b) * NIMGP:(k * B + b + 1) * NIMGP],
                                             start=(k == 0), stop=(k == VK - 1))
                        ev = sb.tile([128, NIMGP], F16, tag="evac", bufs=2, name=f"pje{g}{mm}{b}")
                        nc.scalar.activation(ev[:], pt[:], AF.Identity,
                                             bias=projb_sb[:, m:m + 1])
                        nc.scalar.dma_start(h_init[b][:, m, 0:NIMG], ev[:, 0:NIMG])

            # =========================================================
            def bcast_pe32(row_ap, nm):
                """[1,CH] f32 row -> [128,CH] f32 SBUF via fp32 PE outer product."""
                bc = sb.tile([128, CH], F32, tag="rbc", bufs=2, name=f"b32{nm}")
                pb = ps.tile([128, CH], F32, tag="psB", bufs=1, name=f"p32{nm}")
                nc.tensor.matmul(pb[:], onesrow32_sb[:], row_ap, start=True, stop=True)
                nc.scalar.activation(bc[:], pb[:], AF.Copy)
                return bc

            def bcast_pe(row_ap, width, nm):
                """[1,width] fp16 row -> [128,width] fp16 SBUF via PE outer product."""
                bc = sb.tile([128, width], F16, tag="bc" if width == S else "rbc",
                             bufs=2, name=f"bct{nm}")
                for c0 in range(0, width, CH):
                    w = min(CH, width - c0)
                    pb = ps.tile([128, CH], F32, tag="psB", bufs=1, name=f"pb{nm}{c0}")
                    nc.tensor.matmul(pb[:, :w], onesrow_sb[:], row_ap[0:1, c0:c0 + w],
                                     start=True, stop=True)
                    nc.scalar.activation(bc[:, c0:c0 + w], pb[:, :w], AF.Copy)
                return bc

            def fill_dma(b, sid, src):
                """Enqueue the xmega load DMAs (scalar ring) for the current h.
                src: ('init', tile) or ('ar', [chunk0, chunk1])."""
                xmega = sb.tile([128, KT * S], F16, tag="xmega", bufs=2,
                                name=f"x{sid}{b}")
                GK = 2                       # k-tiles per slab
                kind, st = src
                for kg in range(KT // GK):
                    k0 = kg * GK
                    xsl = xmega[:, k0 * S:(k0 + GK) * S]
                    if kind == 'init':
                        nc.scalar.dma_start(xsl, st[:, k0:k0 + GK, :])
                    else:
                        cix = k0 // KC
                        nc.scalar.dma_start(xsl, st[cix][:, k0 - cix * KC:k0 - cix * KC + GK, :])
                return xmega

            def fill_norm(b, sid, xmega):
                """Square + col-packed partition-sum matmuls -> 1/rms broadcast."""
                ssq_ps = [ps.tile([128, CH], F32, tag="psA", bufs=2, name=f"sq{sid}{b}{c}")
                          for c in range(NCH)]
                for kidx in range(KT):
                    r4 = kidx % 4
                    sq = sb.tile([128, S], F16, tag="sq16", bufs=4,
                                 name=f"s{sid}{b}{kidx}")
                    xk = xmega[:, kidx * S:(kidx + 1) * S]
                    nc.vector.tensor_tensor(sq[:], xk, xk, ALU.mult)
                    for c in range(NCH):
                        nc.tensor.matmul(ssq_ps[c][32 * r4:32 * r4 + 1, :], onesb_sb[:],
                                         sq[:, c * CH:(c + 1) * CH],
                                         start=(kidx < 4), stop=(kidx >= KT - 4),
                                         tile_position=(0, 32 * r4),
                                         skip_group_check=True)
                s_sb = sb.tile([1, S], F16, tag="scal", bufs=2, name=f"ss{sid}{b}")
                r_sb = sb.tile([1, S], F16, tag="scal2", bufs=2, name=f"sr{sid}{b}")
                for c in range(NCH):
                    p4 = sb.tile([128, CH], F32R, tag="p4", bufs=2, name=f"p4{sid}{b}{c}")
                    nc.vector.memset(p4[:].bitcast(F32), 0.0)
                    for r4 in range(4):
                        nc.scalar.activation(p4[32 * r4:32 * r4 + 1, :],
                                             ssq_ps[c][32 * r4:32 * r4 + 1, :], AF.Copy)
                    s1 = ps.tile([1, CH], F32, tag="psS", bufs=2, name=f"s1{sid}{b}{c}")
                    nc.tensor.matmul(s1[:], sel4_sb[0:97, :], p4[0:97, :],
                                     start=True, stop=True)
                    nc.scalar.activation(s_sb[:, c * CH:(c + 1) * CH], s1[:],
                                         AF.Sqrt, scale=1.0 / D, bias=eps_sb[0:1, :])
                with nc.allow_low_precision(reason="fp16 enough for 1/rms scales"):
                    nc.vector.reciprocal(r_sb[:], s_sb[:])
                bc = bcast_pe(r_sb[:], S, f"n{sid}{b}")
                return bc

            def qkv_attn(b, l, xmega, bc):
                """QKV + rope + attention -> amega (128, QH*S) fp16 resident."""
                qk_sb = {}
                vsb = sb.tile([128, S], F16, tag="vsb", bufs=1, name=f"v{l}{b}")
                for m in range(6):
                    sl = sb.tile([128, KT * 128], F16, tag="wbig", bufs=2, name=f"sq{l}{b}{m}")
                    nc.sync.dma_start(sl[:], wqkv_in[l][m])
                    if m < 5:
                        qraw = sb.tile([128, S], F16, tag="rope", bufs=3, name=f"qr{l}{b}{m}")
                    for c in range(NCH):
                        cs_ = slice(c * CH, (c + 1) * CH)
                        pt = ps.tile([128, CH], F32, tag="ps1", bufs=3, name=f"qp{l}{b}{m}{c}")
                        for k in range(KT):
                            nc.tensor.matmul(
                                pt[:], sl[:, k * 128:(k + 1) * 128],
                                xmega[:, k * S + c * CH: k * S + (c + 1) * CH],
                                start=(k == 0), stop=(k == KT - 1))
                        if m < 5:
                            nc.scalar.activation(qraw[:, cs_], pt[:], AF.Copy)
                        else:
                            nc.vector.tensor_tensor(vsb[:, cs_], pt[:], bc[:, cs_], ALU.mult)
                    if m < 5:
                        qs = sb.tile([128, S], F16, tag="rope", bufs=3, name=f"qh{l}{b}{m}")
                        nc.scalar.dma_start(qs[0:64, :], qraw[64:128, :])
                        nc.scalar.dma_start(qs[64:128, :], qraw[0:64, :])
                        t2 = sb.tile([128, S], F16, tag="rope", bufs=3, name=f"t2{l}{b}{m}")
                        nc.vector.tensor_tensor(t2[:], qraw[:], cos_sb[:], ALU.mult)
                        u2 = sb.tile([128, S], F16, tag="rope", bufs=3, name=f"u2{l}{b}{m}")
                        nc.vector.tensor_tensor(u2[:], qs[:], sin_sb[:], ALU.mult)
                        q3 = sb.tile([128, S], F16, tag="rope", bufs=3, name=f"q3{l}{b}{m}")
                        nc.vector.tensor_tensor(q3[:], t2[:], u2[:], ALU.add)
                        qf = sb.tile([128, S], F16, tag="qk5", bufs=5, name=f"qf{l}{b}{m}")
                        nc.vector.tensor_tensor(qf[:], q3[:], bc[:], ALU.mult)
                        qk_sb[m] = qf

                vtok = []
                for t in range(6):
                    trp = ps.tile([128, 128], F16, tag="ps1", bufs=3, name=f"vt{l}{b}{t}")
                    nc.tensor.transpose(trp[:], vsb[:, t * 128:(t + 1) * 128], ident_sb[:])
                    vt = sb.tile([128, 128], F16, tag="vtok", bufs=6, name=f"vk{l}{b}{t}")
                    nc.scalar.activation(vt[:], trp[:], AF.Copy)
                    vtok.append(vt)

                amega = sb.tile([128, QH * S], F16, tag="amega", bufs=1, name=f"am{l}{b}")
                ksb = qk_sb[4]
                for hh in range(QH):
                    qh_t = qk_sb[hh]
                    for c in range(NCH):
                        njt = 3 * (c + 1)
                        ap_ps = ps.tile([128, CH], F32, tag="psA", bufs=2, name=f"ap{l}{b}{hh}{c}")
                        ss_ps = ps.tile([1, CH], F32, tag="psS", bufs=2, name=f"sm{l}{b}{hh}{c}")
                        for jt in range(njt):
                            sc = ps.tile([128, CH], F32, tag="ps1", bufs=3, name=f"sc{l}{b}{hh}{c}{jt}")
                            nc.tensor.matmul(sc[:], ksb[:, jt * 128:(jt + 1) * 128],
                                             qh_t[:, c * CH:(c + 1) * CH],
                                             start=True, stop=True)
                            et = sb.tile([128, CH], F16, tag="expT", bufs=3, name=f"et{l}{b}{hh}{c}{jt}")
                            if jt >= 3 * c:
                                off = 256 - 128 * (jt - 3 * c)
                                madd = sb.tile([128, CH], F32, tag="madd", bufs=2, name=f"md{l}{b}{hh}{c}{jt}")
                                nc.vector.tensor_tensor(madd[:], sc[:], mask_sb[:, off:off + CH], ALU.add)
                                nc.scalar.activation(et[:], madd[:], AF.Exp, scale=ISQ, bias=nb_sb[:])
                            else:
                                nc.scalar.activation(et[:], sc[:], AF.Exp, scale=ISQ, bias=nb_sb[:])
                            nc.tensor.matmul(ss_ps[:], onesb_sb[:], et[:],
                                             start=(jt == 0), stop=(jt == njt - 1))
                            nc.tensor.matmul(ap_ps[:], vtok[jt][:], et[:],
                                             start=(jt == 0), stop=(jt == njt - 1))
                        rec = sb.tile([1, CH], F32, tag="scal", bufs=2, name=f"rc{l}{b}{hh}{c}")
                        nc.vector.reciprocal(rec[:], ss_ps[:])
                        rbc = bcast_pe32(rec[:], f"a{l}{b}{hh}{c}")
                        nc.vector.tensor_tensor(
                            amega[:, hh * S + c * CH: hh * S + (c + 1) * CH],
                            ap_ps[:], rbc[:], ALU.mult)
                return amega

            def start_ar(arin, sid):
                arout = []
                for cix in range(2):
                    ao = dram.tile([128, KC, S], F16, tag="arout", bufs=4,
                                   addr_space="Shared", name=f"ao{sid}{cix}")
                    nc.gpsimd.collective_compute("AllReduce", ALU.add, replica_groups=RG,
                                                 ins=[arin[cix].opt()], outs=[ao.opt()])
                    arout.append(ao)
                return arout

            def wo_site(b, l, amega, xmega):
                """Wo row-parallel partial + h/8 + chunked fp16 AllReduce -> new h."""
                arin = [dram.tile([128, KC, S], F16, tag="arin", bufs=4, name=f"ai{l}o{b}{cix}")
                        for cix in range(2)]
                for g in range(KT // 4):
                    sl = sb.tile([128, 4 * QH * 128], F16, tag="wbig", bufs=2, name=f"so{l}{b}{g}")
                    nc.sync.dma_start(sl[:], wo_in[l][g])
                    for half in range(2):
                        ev = sb.tile([128, 2, S], F16, tag="evac", bufs=2, name=f"oe{l}{b}{g}{half}")
                        for mm2 in range(2):
                            mm = half * 2 + mm2
                            mt = 4 * g + 2 * half + mm2      # global k-tile index of this output row-tile
                            for c in range(NCH):
                                pt = ps.tile([128, CH], F32, tag="ps1", bufs=3,
                                             name=f"o{l}{b}{g}{mm}{c}")
                                for k in range(QH):
                                    nc.tensor.matmul(
                                        pt[:], sl[:, (mm * QH + k) * 128:(mm * QH + k + 1) * 128],
                                        amega[:, k * S + c * CH: k * S + (c + 1) * CH],
                                        start=(k == 0), stop=(k == QH - 1))
                                nc.vector.scalar_tensor_tensor(
                                    ev[:, mm2, c * CH:(c + 1) * CH],
                                    xmega[:, mt * S + c * CH: mt * S + (c + 1) * CH],
                                    1.0 / NCORES, pt[:], ALU.mult, ALU.add)
                        k0 = 4 * g + 2 * half
                        cix = k0 // KC
                        nc.scalar.dma_start(arin[cix][:, k0 - cix * KC:k0 - cix * KC + 2, :], ev[:])
                return start_ar(arin, f"{l}o{b}")

            def mlp_site(b, l, xmega, bc):
                """gate/up gemms + silu*u + single-pass down-proj + h/8 + chunked AR."""
                arin = [dram.tile([128, KC, S], F16, tag="arin", bufs=4, name=f"ai{l}d{b}{cix}")
                        for cix in range(2)]
                mts = {}
                for j in range(FT):
                    gt = sb.tile([128, S], F16, tag="tmp16", bufs=3, name=f"gs{l}{b}{j}")
                    ut = sb.tile([128, S], F16, tag="tmp16", bufs=3, name=f"us{l}{b}{j}")
                    for gu in range(2):
                        sl = sb.tile([128, KT * 128], F16, tag="wbig", bufs=2,
                                     name=f"sg{l}{b}{j}{gu}")
                        nc.sync.dma_start(sl[:], wgu_in[l][j, gu])
                        dst = gt if gu == 0 else ut
                        for c in range(NCH):
                            cs_ = slice(c * CH, (c + 1) * CH)
                            pt = ps.tile([128, CH], F32, tag="ps1", bufs=3,
                                         name=f"g{l}{b}{j}{gu}{c}")
                            for k in range(KT):
                                nc.tensor.matmul(
                                    pt[:], sl[:, k * 128:(k + 1) * 128],
                                    xmega[:, k * S + c * CH: k * S + (c + 1) * CH],
                                    start=(k == 0), stop=(k == KT - 1))
                            nc.vector.tensor_tensor(dst[:, cs_], pt[:], bc[:, cs_], ALU.mult)
                    sil = sb.tile([128, S], F16, tag="tmp16", bufs=3, name=f"si{l}{b}{j}")
                    nc.scalar.activation(sil[:], gt[:], AF_SILU)
                    mt = sb.tile([128, S], F16, tag="mstream", bufs=FT, name=f"mt{l}{b}{j}")
                    nc.vector.tensor_tensor(mt[:], sil[:], ut[:], ALU.mult)
                    mts[j] = mt
                for g in range(KT // 2):
                    sl = sb.tile([128, 2 * FT * 128], F16, tag="wbig", bufs=2,
                                 name=f"sd{l}{b}{g}")
                    nc.sync.dma_start(sl[:], wd_in[l][g])
                    ev = sb.tile([128, 2, S], F16, tag="evac", bufs=2, name=f"de{l}{b}{g}")
                    for mm in range(2):
                        mt_i = 2 * g + mm                    # global k-tile index
                        for c in range(NCH):
                            pt = ps.tile([128, CH], F32, tag="ps1", bufs=3,
                                         name=f"dp{l}{b}{g}{mm}{c}")
                            for k in range(FT):
                                nc.tensor.matmul(
                                    pt[:], sl[:, (mm * FT + k) * 128:(mm * FT + k + 1) * 128],
                                    mts[k][:, c * CH:(c + 1) * CH],
                                    start=(k == 0), stop=(k == FT - 1))
                            nc.vector.scalar_tensor_tensor(
                                ev[:, mm, c * CH:(c + 1) * CH],
                                xmega[:, mt_i * S + c * CH: mt_i * S + (c + 1) * CH],
                                1.0 / NCORES, pt[:], ALU.mult, ALU.add)
                    k0 = 2 * g
                    cix = k0 // KC
                    nc.scalar.dma_start(arin[cix][:, k0 - cix * KC:k0 - cix * KC + 2, :], ev[:])
                return start_ar(arin, f"{l}d{b}")

            def final_norm(b, xmega, bc):
                for kg in range(KT // 2):
                    ot = sb.tile([128, 2, S], F16, tag="evac", bufs=2, name=f"ot{b}{kg}")
                    for kk in range(2):
                        k = kg * 2 + kk
                        nc.vector.scalar_tensor_tensor(
                            ot[:, kk, :], xmega[:, k * S:(k + 1) * S],
                            lnf_sb[:, k:k + 1], bc[:], ALU.mult, ALU.mult)
                    nc.scalar.dma_start(out_ext[:, kg * 2:kg * 2 + 2, b * S:(b + 1) * S], ot[:])

            # ---- main schedule ----
            # fills are split into .dma (scalar-ring enqueue, placed at the
            # earliest dependency-safe slot) and .norm (PE ssq, placed late
            # enough that the AR + load have completed under prior compute).
            srcA = [('init', h_init[0]), ('init', h_init[1])]
            xmA = [fill_dma(0, 'a0', srcA[0]), None]
            for l in range(NL):
                sidA, sidM = f"a{l}", f"m{l}"
                bcA0 = fill_norm(0, sidA, xmA[0])
                am0 = qkv_attn(0, l, xmA[0], bcA0)
                xmA[1] = fill_dma(1, sidA, srcA[1])
                arW0 = wo_site(0, l, am0, xmA[0])
                bcA1 = fill_norm(1, sidA, xmA[1])
                am1 = qkv_attn(1, l, xmA[1], bcA1)
                xmM0 = fill_dma(0, sidM, ('ar', arW0))
                arW1 = wo_site(1, l, am1, xmA[1])
                bcM0 = fill_norm(0, sidM, xmM0)
                xmM1 = fill_dma(1, sidM, ('ar', arW1))
                arM0 = mlp_site(0, l, xmM0, bcM0)
                srcA[0] = ('ar', arM0)
                xmA[0] = fill_dma(0, f"a{l + 1}", srcA[0])
                bcM1 = fill_norm(1, sidM, xmM1)
                arM1 = mlp_site(1, l, xmM1, bcM1)
                srcA[1] = ('ar', arM1)
            xmF1 = fill_dma(1, 'f1', srcA[1])
            bcF0 = fill_norm(0, 'f0', xmA[0])
            final_norm(0, xmA[0], bcF0)
            bcF1 = fill_norm(1, 'f1', xmF1)
            final_norm(1, xmF1, bcF1)

    nc.compile()
    return nc


def _host_prep(inputs):
    I = {k: np.asarray(v) for k, v in inputs.items()}

    def fold(W, A, Bm, lnw=None):
        W64 = W.astype(np.float64) + SCALE * (Bm.astype(np.float64) @ A.astype(np.float64))
        if lnw is not None:
            W64 = W64 * lnw.astype(np.float64)[None, :]
        return W64.astype(np.float32)

    ids = np.asarray(I['input_ids'], np.int64)
    txt = I['embed'][ids]                                    # (B, T, D)
    txtT = txt.reshape(B * T, D).T.astype(np.float32)        # (D, B*T) order: b-major cols
    txt16 = _h(txtT).reshape(KT, 128, B * T).transpose(1, 0, 2)   # [128, KT, B*T]

    inv = 1.0 / (10000.0 ** (np.arange(0, HD, 2, dtype=np.float64) / HD))
    ang = np.arange(S, dtype=np.float64)[:, None] * inv[None, :]
    cosT = _h(np.ascontiguousarray(np.concatenate([np.cos(ang), np.cos(ang)], 1).T))
    sinT = _h(np.ascontiguousarray(np.concatenate([-np.sin(ang), np.sin(ang)], 1).T))

    # one shifted master mask: mbig[j, u] = 0 iff j <= u - 256; slice
    # [:, 256-128*jtl : 640-128*jtl] recovers the per-key-block causal mask
    jj = np.arange(128)[:, None]
    uu = np.arange(640)[None, :]
    mbig = np.where(jj <= uu - 256, 0.0, MASK_NEG).astype(np.float32)

    imgT = I['image_embeds'].reshape(B * NIMG, VH).T.astype(np.float32)   # (VH, B*NIMG)
    imp = np.zeros((VK, 128, B * NIMGP), np.float32)
    for k in range(VK):
        for b in range(B):
            imp[k, :, b * NIMGP:b * NIMGP + NIMG] = imgT[k * 128:(k + 1) * 128,
                                                         b * NIMG:(b + 1) * NIMG]
    img16 = _h(imp.transpose(1, 0, 2).reshape(128, VK * B * NIMGP))

    projT = I['proj_W'].astype(np.float32).T                 # (VH, D)
    pw = projT.reshape(VK, 128, KT, 128).transpose(2, 1, 0, 3)        # [m,p,k,f]
    pw = pw.reshape(KT // 4, 4, 128, VK, 128).transpose(0, 2, 1, 3, 4)
    projw16 = _h(pw.reshape(KT // 4, 128, 4 * VK * 128))

    projb_t = np.ascontiguousarray(I['proj_b'].astype(np.float32).reshape(KT, 128).T)
    lnf_t = np.ascontiguousarray(I['ln_f'].astype(np.float32).reshape(KT, 128).T)

    shared = dict(
        projb=projb_t, txt=np.ascontiguousarray(txt16), cos_t=cosT, sin_t=sinT,
        mbig=_bf(mbig),
        onesb=_h(np.ones((128, 1), np.float32)),
        onesrow=_h(np.ones((1, 128), np.float32)),
        onesrow32=np.ones((1, 128), np.float32),
        sel4=np.ascontiguousarray(
            np.where(np.arange(128)[:, None] % 32 == 0, 1.0, 0.0).astype(np.float32)),
        ident=_h(np.eye(128, dtype=np.float32)),
        lnf=lnf_t,
        img=img16, projw=projw16,
    )

    per_core = [dict(shared) for _ in range(NCORES)]

    for l in range(NL):
        Wq = fold(I['Wq'][l], I['Aq'][l], I['Bq'][l], I['ln1'][l])
        Wk = fold(I['Wk'][l], I['Ak'][l], I['Bk'][l], I['ln1'][l])
        Wv = fold(I['Wv'][l], I['Av'][l], I['Bv'][l], I['ln1'][l])
        Wo = fold(I['Wo'][l], I['Ao'][l], I['Bo'][l])
        Wg = fold(I['Wg'][l], I['Ag'][l], I['Bg'][l], I['ln2'][l])
        Wu = fold(I['Wu'][l], I['Au'][l], I['Bu'][l], I['ln2'][l])
        Wd = fold(I['Wd'][l], I['Ad'][l], I['Bd'][l])
        for r in range(NCORES):
            qs = Wq[r * DSH:(r + 1) * DSH]
            ks = Wk[r * HD:(r + 1) * HD]
            vs = Wv[r * HD:(r + 1) * HD]
            wqkvT = np.vstack([qs, ks, vs]).T                # (D, 768)
            arr = wqkvT.reshape(KT, 128, 6, 128).transpose(2, 1, 0, 3)   # [m,p,k,f]
            per_core[r][f"wqkv{l}"] = _h(arr.reshape(6, 128, KT * 128))

            woT = Wo[:, r * DSH:(r + 1) * DSH].T             # (512, D) rows=k-in, cols=d-out
            arr = woT.reshape(QH, 128, KT, 128).transpose(2, 1, 0, 3)    # [m,p,k,f]
            arr = arr.reshape(KT // 4, 4, 128, QH, 128).transpose(0, 2, 1, 3, 4)
            per_core[r][f"wo{l}"] = _h(arr.reshape(KT // 4, 128, 4 * QH * 128))

            gT = Wg[r * FSH:(r + 1) * FSH].T                 # (D, FSH)
            uT = Wu[r * FSH:(r + 1) * FSH].T
            ga = gT.reshape(KT, 128, FT, 128).transpose(2, 1, 0, 3).reshape(FT, 128, KT * 128)
            ua = uT.reshape(KT, 128, FT, 128).transpose(2, 1, 0, 3).reshape(FT, 128, KT * 128)
            per_core[r][f"wgu{l}"] = _h(np.stack([ga, ua], axis=1))

            wdT = Wd[:, r * FSH:(r + 1) * FSH].T             # (FSH, D)
            arr = wdT.reshape(FT, 128, KT, 128).transpose(2, 1, 0, 3)    # [m,p,j,f]
            arr = arr.reshape(KT // 2, 2, 128, FT, 128).transpose(0, 2, 1, 3, 4)
            per_core[r][f"wd{l}"] = _h(arr.reshape(KT // 2, 128, 2 * FT * 128))
    return per_core


def kernel(**inputs):
    global _PROGRAM
    from concourse.bass_utils import run_bass_kernel_spmd

    in_maps = _host_prep(inputs)
    if _PROGRAM is None:
        _PROGRAM = _build_program()
    res = None
    for attempt in range(3):
        try:
            res = run_bass_kernel_spmd(_PROGRAM, in_maps, list(range(NCORES)))
            break
        except Exception as e:
            if attempt == 2 or 'UNAVAILABLE' not in str(type(e).__name__) + str(e):
                raise
    out = np.asarray(res.results[0]["out"], np.float32)      # [128, KT, B*S]
    full = out.reshape(128, KT, B, S).transpose(2, 3, 1, 0)  # (B, S, KT, 128)
    return np.ascontiguousarray(full.reshape(B, S, D))


# revision 22
# speedup vs baseline: 1.0958x; 1.0958x over previous
"""Trainium2 Bass kernel for a 2-layer Mistral-style VLM block (tensor-parallel, 8 cores).

v3 strategy (on top of v2's fp16 GEMMs / folded LoRA+ln / batch-pipelined AR):
- The AllReduce carries the residual stream: each core adds h/8 to its partial
  (Sum_r(partial_r + h/8) = h + Sum partial), so the AR output IS the new h.
  Kills all per-site h DRAM round-trips (load h + ar, add, write back).
- xmega (the resident activation megatile) is double-buffered so batch b1's
  fill + rmsnorm overlaps batch b0's GEMMs.
- Each AR is split into 2 chunks along k-tiles so the collective starts while
  the producer GEMMs still run and the consumer fill starts on chunk 0 while
  chunk 1 is in flight.
- q/k head tiles stay in SBUF (no DRAM bounce).
- Output written fp16 (host converts to f32), halving the output-write tail.
- Wo/Wd PSUM evacuation moved to DVE scalar_tensor_tensor (adds h/8 in the
  same op), offloading ScalarE which is busy with exp during attention.
"""

import os
import sys

sys.path.insert(0, '/opt/trn_rl_repo')

import numpy as np
import ml_dtypes

NCORES = 8
D, VH, DFF, NL, VOCAB, NH, NKV, HD, RK, SCALE = 4096, 1024, 14336, 2, 32000, 32, 8, 128, 8, 4.0
B, NIMG, T = 2, 257, 511
S = NIMG + T            # 768
NTOK = B * S            # 1536
DSH = D // NCORES       # 512
FSH = DFF // NCORES     # 1792
KT = D // 128           # 32
FT = FSH // 128         # 14
VK = VH // 128          # 8
QH = NH // NCORES       # 4
CH = 384
NCH = S // CH           # 2
EPS = 1e-5
ISQ = 1.0 / float(np.sqrt(HD))
EXP_BIAS = 0.0   # exp(s) <= e^10.8 < 65504 fits fp16; rowsum stays in normal range
MASK_NEG = -1e30
NIMGP = NIMG + 1          # pad to even free size
KC = KT // 2              # 16 k-tiles per AR chunk

BF16 = ml_dtypes.bfloat16
F16NP = np.float16
_PROGRAM = None


def _bf(x):
    return np.ascontiguousarray(np.asarray(x, np.float32).astype(BF16))


def _h(x):
    return np.ascontiguousarray(np.asarray(x, np.float32).astype(F16NP))


def _build_program():
    import concourse.bass as bass
    import concourse.bacc as bacc
    import concourse.mybir as mybir
    import concourse.tile as tile

    F32 = mybir.dt.float32
    F32R = mybir.dt.float32r
    F16 = mybir.dt.float16
    BF = mybir.dt.bfloat16
    AF = mybir.ActivationFunctionType
    ALU = mybir.AluOpType
    AF_SILU = AF.Sigmoid if os.environ.get('KSIM') == '1' else AF.Silu

    nc = bacc.Bacc("TRN2", target_bir_lowering=False)

    img_in = nc.dram_tensor("img", [128, VK * B * NIMGP], F16, kind="ExternalInput")
    projw_in = nc.dram_tensor("projw", [KT // 4, 128, 4 * VK * 128], F16, kind="ExternalInput")
    projb_in = nc.dram_tensor("projb", [128, KT], F32, kind="ExternalInput")
    txt_in = nc.dram_tensor("txt", [128, KT, B * T], F16, kind="ExternalInput")
    cos_in = nc.dram_tensor("cos_t", [128, S], F16, kind="ExternalInput")
    sin_in = nc.dram_tensor("sin_t", [128, S], F16, kind="ExternalInput")   # sign-folded
    mask_in = nc.dram_tensor("mbig", [128, 640], BF, kind="ExternalInput")
    onesb_in = nc.dram_tensor("onesb", [128, 1], F16, kind="ExternalInput")
    onesrow_in = nc.dram_tensor("onesrow", [1, 128], F16, kind="ExternalInput")
    onesrow32_in = nc.dram_tensor("onesrow32", [1, 128], F32, kind="ExternalInput")
    sel4_in = nc.dram_tensor("sel4", [128, 1], mybir.dt.float32r, kind="ExternalInput")
    ident_in = nc.dram_tensor("ident", [128, 128], F16, kind="ExternalInput")
    lnf_in = nc.dram_tensor("lnf", [128, KT], F32, kind="ExternalInput")
    wqkv_in = [nc.dram_tensor(f"wqkv{l}", [6, 128, KT * 128], F16, kind="ExternalInput") for l in range(NL)]
    wo_in = [nc.dram_tensor(f"wo{l}", [KT // 4, 128, 4 * QH * 128], F16, kind="ExternalInput") for l in range(NL)]
    wgu_in = [nc.dram_tensor(f"wgu{l}", [FT, 2, 128, KT * 128], F16, kind="ExternalInput") for l in range(NL)]
    wd_in = [nc.dram_tensor(f"wd{l}", [KT // 2, 128, 2 * FT * 128], F16, kind="ExternalInput") for l in range(NL)]
    out_ext = nc.dram_tensor("out", [128, KT, NTOK], F16, kind="ExternalOutput")

    RG = [list(range(NCORES))]

    with tile.TileContext(nc) as tc:
        with tc.tile_pool(name="sb", bufs=1) as sb, \
             tc.tile_pool(name="ps", bufs=1, space="PSUM") as ps, \
             tc.tile_pool(name="dram", bufs=1, space="DRAM") as dram:

            # ---- resident constants ----
            cos_sb = sb.tile([128, S], F16, tag="res_cos", bufs=1)
            sin_sb = sb.tile([128, S], F16, tag="res_sin", bufs=1)
            onesb_sb = sb.tile([128, 1], F16, tag="res_onesb", bufs=1)
            onesrow_sb = sb.tile([1, 128], F16, tag="res_onesrow", bufs=1)
            onesrow32_sb = sb.tile([1, 128], F32, tag="res_onesrow32", bufs=1)
            ident_sb = sb.tile([128, 128], F16, tag="res_ident", bufs=1)
            projb_sb = sb.tile([128, KT], F32, tag="res_projb", bufs=1)
            lnf_sb = sb.tile([128, KT], F32, tag="res_lnf", bufs=1)
            for t_, i_ in [(cos_sb, cos_in), (sin_sb, sin_in), (onesb_sb, onesb_in),
                           (onesrow_sb, onesrow_in), (onesrow32_sb, onesrow32_in),
                           (ident_sb, ident_in),
                           (projb_sb, projb_in), (lnf_sb, lnf_in)]:
                nc.sync.dma_start(t_[:], i_[:])
            mask_sb = sb.tile([128, 640], BF, tag="res_mask", bufs=1, name="mskbig")
            nc.sync.dma_start(mask_sb[:], mask_in[:])
            sel4_sb = sb.tile([128, 1], F32R, tag="res_sel4", bufs=1)
            nc.sync.dma_start(sel4_sb[:], sel4_in[:])
            eps_sb = sb.tile([128, 1], F32, tag="res_eps", bufs=1)
            nb_sb = sb.tile([128, 1], F32, tag="res_nb", bufs=1)
            nc.vector.memset(eps_sb[:], EPS)
            nc.vector.memset(nb_sb[:], EXP_BIAS)

            # phase-0 output: initial h per batch (img tokens from projector,
            # txt tokens DMA'd straight from host-prepped embedding rows)
            h_init = [dram.tile([128, KT, S], F16, tag=f"hinit{b}", bufs=1, name=f"h_i{b}")
                      for b in range(B)]

            # ---- phase 0: replicated projector (no collective); txt DMA on the
            # scalar HWDGE ring so it does not block img/weight loads on sync ----
            img_sb = sb.tile([128, VK * B * NIMGP], F16, tag="imgt", bufs=1, name="img_sb")
            nc.sync.dma_start(img_sb[:], img_in[:])
            for b in range(B):
                nc.scalar.dma_start(h_init[b][:, :, NIMG:S], txt_in[:, :, b * T:(b + 1) * T])
            for g in range(KT // 4):
                pw = sb.tile([128, 4 * VK * 128], F16, tag="wbig", bufs=2, name=f"pw{g}")
                nc.sync.dma_start(pw[:], projw_in[g])
                for mm in range(4):
                    m = 4 * g + mm
                    for b in range(B):
                        pt = ps.tile([128, NIMGP], F32, tag="ps1", bufs=4, name=f"pj{g}{mm}{b}")
                        for k in range(VK):
                            nc.tensor.matmul(pt[:], pw[:, (mm * VK + k) * 128:(mm * VK + k + 1) * 128],
                                             img_sb[:, (k * B + b) * NIMGP:(k * B + b + 1) * NIMGP],
                                             start=(k == 0), stop=(k == VK - 1))
                        ev = sb.tile([128, NIMGP], F16, tag="evac", bufs=2, name=f"pje{g}{mm}{b}")
                        nc.scalar.activation(ev[:], pt[:], AF.Identity,
                                             bias=projb_sb[:, m:m + 1])
                        nc.scalar.dma_start(h_init[b][:, m, 0:NIMG], ev[:, 0:NIMG])

            # =========================================================
            def bcast_pe32(row_ap, nm):
                """[1,CH] f32 row -> [128,CH] f32 SBUF via fp32 PE outer product."""
                bc = sb.tile([128, CH], F32, tag="rbc", bufs=2, name=f"b32{nm}")
                pb = ps.tile([128, CH], F32, tag="psB", bufs=1, name=f"p32{nm}")
                nc.tensor.matmul(pb[:], onesrow32_sb[:], row_ap, start=True, stop=True)
                nc.scalar.activation(bc[:], pb[:], AF.Copy)
                return bc

            def bcast_pe(row_ap, width, nm):
                """[1,width] fp16 row -> [128,width] fp16 SBUF via PE outer product."""
                bc = sb.tile([128, width], F16, tag="bc" if width == S else "rbc",
                             bufs=2, name=f"bct{nm}")
                for c0 in range(0, width, CH):
                    w = min(CH, width - c0)
                    pb = ps.tile([128, CH], F32, tag="psB", bufs=1, name=f"pb{nm}{c0}")
                    nc.tensor.matmul(pb[:, :w], onesrow_sb[:], row_ap[0:1, c0:c0 + w],
                                     start=True, stop=True)
                    nc.scalar.activation(bc[:, c0:c0 + w], pb[:, :w], AF.Copy)
                return bc

            def fill_dma(b, sid, src):
                """Enqueue the xmega load DMAs (scalar ring) for the current h.
                src: ('init', tile) or ('ar', [chunk0, chunk1])."""
                xmega = sb.tile([128, KT * S], F16, tag="xmega", bufs=2,
                                name=f"x{sid}{b}")
                GK = 2                       # k-tiles per slab
                kind, st = src
                for kg in range(KT // GK):
                    k0 = kg * GK
                    xsl = xmega[:, k0 * S:(k0 + GK) * S]
                    if kind == 'init':
                        nc.scalar.dma_start(xsl, st[:, k0:k0 + GK, :])
                    else:
                        cix = k0 // KC
                        nc.scalar.dma_start(xsl, st[cix][:, k0 - cix * KC:k0 - cix * KC + GK, :])
                return xmega

            def fill_norm(b, sid, xmega):
                """Square + col-packed partition-sum matmuls -> 1/rms broadcast."""
                ssq_ps = [ps.tile([128, CH], F32, tag="psA", bufs=2, name=f"sq{sid}{b}{c}")
                          for c in range(NCH)]
                for kidx in range(KT):
                    r4 = kidx % 4
                    sq = sb.tile([128, S], F16, tag="sq16", bufs=4,
                                 name=f"s{sid}{b}{kidx}")
                    xk = xmega[:, kidx * S:(kidx + 1) * S]
                    nc.vector.tensor_tensor(sq[:], xk, xk, ALU.mult)
                    for c in range(NCH):
                        nc.tensor.matmul(ssq_ps[c][32 * r4:32 * r4 + 1, :], onesb_sb[:],
                                         sq[:, c * CH:(c + 1) * CH],
                                         start=(kidx < 4), stop=(kidx >= KT - 4),
                                         tile_position=(0, 32 * r4),
                                         skip_group_check=True)
                s_sb = sb.tile([1, S], F16, tag="scal", bufs=2, name=f"ss{sid}{b}")
                r_sb = sb.tile([1, S], F16, tag="scal2", bufs=2, name=f"sr{sid}{b}")
                for c in range(NCH):
                    p4 = sb.tile([128, CH], F32R, tag="p4", bufs=2, name=f"p4{sid}{b}{c}")
                    nc.vector.memset(p4[:].bitcast(F32), 0.0)
                    for r4 in range(4):
                        nc.scalar.activation(p4[32 * r4:32 * r4 + 1, :],
                                             ssq_ps[c][32 * r4:32 * r4 + 1, :], AF.Copy)
                    s1 = ps.tile([1, CH], F32, tag="psS", bufs=1, name=f"s1{sid}{b}{c}")
                    nc.tensor.matmul(s1[:], sel4_sb[0:97, :], p4[0:97, :],
                                     start=True, stop=True)
                    nc.scalar.activation(s_sb[:, c * CH:(c + 1) * CH], s1[:],
                                         AF.Sqrt, scale=1.0 / D, bias=eps_sb[0:1, :])
                with nc.allow_low_precision(reason="fp16 enough for 1/rms scales"):
                    nc.vector.reciprocal(r_sb[:], s_sb[:])
                bc = bcast_pe(r_sb[:], S, f"n{sid}{b}")
                return bc

            def qkv_attn(b, l, xmega, bc):
                """QKV + rope + attention -> amega (128, QH*S) fp16 resident."""
                qk_sb = {}
                vsb = sb.tile([128, S], F16, tag="vsb", bufs=1, name=f"v{l}{b}")
                for m in range(6):
                    sl = sb.tile([128, KT * 128], F16, tag="wbig", bufs=2, name=f"sq{l}{b}{m}")
                    nc.sync.dma_start(sl[:], wqkv_in[l][m])
                    if m < 5:
                        qraw = sb.tile([128, S], F16, tag="rope", bufs=3, name=f"qr{l}{b}{m}")
                    for c in range(NCH):
                        cs_ = slice(c * CH, (c + 1) * CH)
                        pt = ps.tile([128, CH], F32, tag="ps1", bufs=4, name=f"qp{l}{b}{m}{c}")
                        for k in range(KT):
                            nc.tensor.matmul(
                                pt[:], sl[:, k * 128:(k + 1) * 128],
                                xmega[:, k * S + c * CH: k * S + (c + 1) * CH],
                                start=(k == 0), stop=(k == KT - 1))
                        if m < 5:
                            nc.scalar.activation(qraw[:, cs_], pt[:], AF.Copy)
                        else:
                            nc.vector.tensor_tensor(vsb[:, cs_], pt[:], bc[:, cs_], ALU.mult)
                    if m < 5:
                        qs = sb.tile([128, S], F16, tag="rope", bufs=3, name=f"qh{l}{b}{m}")
                        nc.scalar.dma_start(qs[0:64, :], qraw[64:128, :])
                        nc.scalar.dma_start(qs[64:128, :], qraw[0:64, :])
                        t2 = sb.tile([128, S], F16, tag="rope", bufs=3, name=f"t2{l}{b}{m}")
                        nc.vector.tensor_tensor(t2[:], qraw[:], cos_sb[:], ALU.mult)
                        u2 = sb.tile([128, S], F16, tag="rope", bufs=3, name=f"u2{l}{b}{m}")
                        nc.vector.tensor_tensor(u2[:], qs[:], sin_sb[:], ALU.mult)
                        q3 = sb.tile([128, S], F16, tag="rope", bufs=3, name=f"q3{l}{b}{m}")
                        nc.vector.tensor_tensor(q3[:], t2[:], u2[:], ALU.add)
                        qf = sb.tile([128, S], F16, tag="qk5", bufs=5, name=f"qf{l}{b}{m}")
                        nc.vector.tensor_tensor(qf[:], q3[:], bc[:], ALU.mult)
                        qk_sb[m] = qf

                vtok = []
                for t in range(6):
                    trp = ps.tile([128, 128], F16, tag="ps1", bufs=4, name=f"vt{l}{b}{t}")
                    nc.tensor.transpose(trp[:], vsb[:, t * 128:(t + 1) * 128], ident_sb[:])
                    vt = sb.tile([128, 128], F16, tag="vtok", bufs=6, name=f"vk{l}{b}{t}")
                    nc.scalar.activation(vt[:], trp[:], AF.Copy)
                    vtok.append(vt)

                amega = sb.tile([128, QH * S], F16, tag="amega", bufs=1, name=f"am{l}{b}")
                ksb = qk_sb[4]
                for hh in range(QH):
                    qh_t = qk_sb[hh]
                    for c in range(NCH):
                        njt = 3 * (c + 1)
                        ap_ps = ps.tile([128, CH], F32, tag="psA", bufs=2, name=f"ap{l}{b}{hh}{c}")
                        ss_ps = ps.tile([1, CH], F32, tag="psS", bufs=1, name=f"sm{l}{b}{hh}{c}")
                        for jt in range(njt):
                            sc = ps.tile([128, CH], F32, tag="ps1", bufs=4, name=f"sc{l}{b}{hh}{c}{jt}")
                            nc.tensor.matmul(sc[:], ksb[:, jt * 128:(jt + 1) * 128],
                                             qh_t[:, c * CH:(c + 1) * CH],
                                             start=True, stop=True)
                            et = sb.tile([128, CH], F16, tag="expT", bufs=4, name=f"et{l}{b}{hh}{c}{jt}")
                            if jt >= 3 * c:
                                off = 256 - 128 * (jt - 3 * c)
                                madd = sb.tile([128, CH], F32, tag="madd", bufs=2, name=f"md{l}{b}{hh}{c}{jt}")
                                nc.vector.tensor_tensor(madd[:], sc[:], mask_sb[:, off:off + CH], ALU.add)
                                nc.scalar.activation(et[:], madd[:], AF.Exp, scale=ISQ, bias=nb_sb[:])
                            else:
                                nc.scalar.activation(et[:], sc[:], AF.Exp, scale=ISQ, bias=nb_sb[:])
                            nc.tensor.matmul(ss_ps[:], onesb_sb[:], et[:],
                                             start=(jt == 0), stop=(jt == njt - 1))
                            nc.tensor.matmul(ap_ps[:], vtok[jt][:], et[:],
                                             start=(jt == 0), stop=(jt == njt - 1))
                        rec = sb.tile([1, CH], F32, tag="scal", bufs=2, name=f"rc{l}{b}{hh}{c}")
                        nc.vector.reciprocal(rec[:], ss_ps[:])
                        rbc = bcast_pe32(rec[:], f"a{l}{b}{hh}{c}")
                        nc.vector.tensor_tensor(
                            amega[:, hh * S + c * CH: hh * S + (c + 1) * CH],
                            ap_ps[:], rbc[:], ALU.mult)
                return amega

            def start_ar(arin, sid):
                arout = []
                for cix in range(2):
                    ao = dram.tile([128, KC, S], F16, tag="arout", bufs=4,
                                   addr_space="Shared", name=f"ao{sid}{cix}")
                    nc.gpsimd.collective_compute("AllReduce", ALU.add, replica_groups=RG,
                                                 ins=[arin[cix].opt()], outs=[ao.opt()])
                    arout.append(ao)
                return arout

            def wo_site(b, l, amega, xmega):
                """Wo row-parallel partial + h/8 + chunked fp16 AllReduce -> new h."""
                arin = [dram.tile([128, KC, S], F16, tag="arin", bufs=4, name=f"ai{l}o{b}{cix}")
                        for cix in range(2)]
                for g in range(KT // 4):
                    sl = sb.tile([128, 4 * QH * 128], F16, tag="wbig", bufs=2, name=f"so{l}{b}{g}")
                    nc.sync.dma_start(sl[:], wo_in[l][g])
                    for half in range(2):
                        ev = sb.tile([128, 2, S], F16, tag="evac", bufs=2, name=f"oe{l}{b}{g}{half}")
                        for mm2 in range(2):
                            mm = half * 2 + mm2
                            mt = 4 * g + 2 * half + mm2      # global k-tile index of this output row-tile
                            for c in range(NCH):
                                pt = ps.tile([128, CH], F32, tag="ps1", bufs=4,
                                             name=f"o{l}{b}{g}{mm}{c}")
                                for k in range(QH):
                                    nc.tensor.matmul(
                                        pt[:], sl[:, (mm * QH + k) * 128:(mm * QH + k + 1) * 128],
                                        amega[:, k * S + c * CH: k * S + (c + 1) * CH],
                                        start=(k == 0), stop=(k == QH - 1))
                                nc.vector.scalar_tensor_tensor(
                                    ev[:, mm2, c * CH:(c + 1) * CH],
                                    xmega[:, mt * S + c * CH: mt * S + (c + 1) * CH],
                                    1.0 / NCORES, pt[:], ALU.mult, ALU.add)
                        k0 = 4 * g + 2 * half
                        cix = k0 // KC
                        nc.scalar.dma_start(arin[cix][:, k0 - cix * KC:k0 - cix * KC + 2, :], ev[:])
                return start_ar(arin, f"{l}o{b}")

            def mlp_site(b, l, xmega, bc):
                """gate/up gemms + silu*u + single-pass down-proj + h/8 + chunked AR."""
                arin = [dram.tile([128, KC, S], F16, tag="arin", bufs=4, name=f"ai{l}d{b}{cix}")
                        for cix in range(2)]
                mts = {}
                for j in range(FT):
                    gt = sb.tile([128, S], F16, tag="tmp16", bufs=3, name=f"gs{l}{b}{j}")
                    ut = sb.tile([128, S], F16, tag="tmp16", bufs=3, name=f"us{l}{b}{j}")
                    for gu in range(2):
                        sl = sb.tile([128, KT * 128], F16, tag="wbig", bufs=2,
                                     name=f"sg{l}{b}{j}{gu}")
                        nc.sync.dma_start(sl[:], wgu_in[l][j, gu])
                        dst = gt if gu == 0 else ut
                        for c in range(NCH):
                            cs_ = slice(c * CH, (c + 1) * CH)
                            pt = ps.tile([128, CH], F32, tag="ps1", bufs=4,
                                         name=f"g{l}{b}{j}{gu}{c}")
                            for k in range(KT):
                                nc.tensor.matmul(
                                    pt[:], sl[:, k * 128:(k + 1) * 128],
                                    xmega[:, k * S + c * CH: k * S + (c + 1) * CH],
                                    start=(k == 0), stop=(k == KT - 1))
                            nc.vector.tensor_tensor(dst[:, cs_], pt[:], bc[:, cs_], ALU.mult)
                    sil = sb.tile([128, S], F16, tag="tmp16", bufs=3, name=f"si{l}{b}{j}")
                    nc.scalar.activation(sil[:], gt[:], AF_SILU)
                    mt = sb.tile([128, S], F16, tag="mstream", bufs=FT, name=f"mt{l}{b}{j}")
                    nc.vector.tensor_tensor(mt[:], sil[:], ut[:], ALU.mult)
                    mts[j] = mt
                for g in range(KT // 2):
                    sl = sb.tile([128, 2 * FT * 128], F16, tag="wbig", bufs=2,
                                 name=f"sd{l}{b}{g}")
                    nc.sync.dma_start(sl[:], wd_in[l][g])
                    ev = sb.tile([128, 2, S], F16, tag="evac", bufs=2, name=f"de{l}{b}{g}")
                    for mm in range(2):
                        mt_i = 2 * g + mm                    # global k-tile index
                        for c in range(NCH):
                            pt = ps.tile([128, CH], F32, tag="ps1", bufs=4,
                                         name=f"dp{l}{b}{g}{mm}{c}")
                            for k in range(FT):
                                nc.tensor.matmul(
                                    pt[:], sl[:, (mm * FT + k) * 128:(mm * FT + k + 1) * 128],
                                    mts[k][:, c * CH:(c + 1) * CH],
                                    start=(k == 0), stop=(k == FT - 1))
                            nc.vector.scalar_tensor_tensor(
                                ev[:, mm, c * CH:(c + 1) * CH],
                                xmega[:, mt_i * S + c * CH: mt_i * S + (c + 1) * CH],
                                1.0 / NCORES, pt[:], ALU.mult, ALU.add)
                    k0 = 2 * g
                    cix = k0 // KC
                    nc.scalar.dma_start(arin[cix][:, k0 - cix * KC:k0 - cix * KC + 2, :], ev[:])
                return start_ar(arin, f"{l}d{b}")

            def final_norm(b, xmega, bc):
                for kg in range(KT // 2):
                    ot = sb.tile([128, 2, S], F16, tag="evac", bufs=2, name=f"ot{b}{kg}")
                    for kk in range(2):
                        k = kg * 2 + kk
                        nc.vector.scalar_tensor_tensor(
                            ot[:, kk, :], xmega[:, k * S:(k + 1) * S],
                            lnf_sb[:, k:k + 1], bc[:], ALU.mult, ALU.mult)
                    nc.scalar.dma_start(out_ext[:, kg * 2:kg * 2 + 2, b * S:(b + 1) * S], ot[:])

            # ---- main schedule ----
            # fills are split into .dma (scalar-ring enqueue, placed at the
            # earliest dependency-safe slot) and .norm (PE ssq, placed late
            # enough that the AR + load have completed under prior compute).
            srcA = [('init', h_init[0]), ('init', h_init[1])]
            xmA = [fill_dma(0, 'a0', srcA[0]), None]
            for l in range(NL):
                sidA, sidM = f"a{l}", f"m{l}"
                bcA0 = fill_norm(0, sidA, xmA[0])
                am0 = qkv_attn(0, l, xmA[0], bcA0)
                xmA[1] = fill_dma(1, sidA, srcA[1])
                arW0 = wo_site(0, l, am0, xmA[0])
                bcA1 = fill_norm(1, sidA, xmA[1])
                am1 = qkv_attn(1, l, xmA[1], bcA1)
                xmM0 = fill_dma(0, sidM, ('ar', arW0))
                arW1 = wo_site(1, l, am1, xmA[1])
                bcM0 = fill_norm(0, sidM, xmM0)
                xmM1 = fill_dma(1, sidM, ('ar', arW1))
                arM0 = mlp_site(0, l, xmM0, bcM0)
                srcA[0] = ('ar', arM0)
                xmA[0] = fill_dma(0, f"a{l + 1}", srcA[0])
                bcM1 = fill_norm(1, sidM, xmM1)
                arM1 = mlp_site(1, l, xmM1, bcM1)
                srcA[1] = ('ar', arM1)
            xmF1 = fill_dma(1, 'f1', srcA[1])
            bcF0 = fill_norm(0, 'f0', xmA[0])
            final_norm(0, xmA[0], bcF0)
            bcF1 = fill_norm(1, 'f1', xmF1)
            final_norm(1, xmF1, bcF1)

    nc.compile()
    return nc


def _host_prep(inputs):
    I = {k: np.asarray(v) for k, v in inputs.items()}

    def fold(W, A, Bm, lnw=None):
        W64 = W.astype(np.float64) + SCALE * (Bm.astype(np.float64) @ A.astype(np.float64))
        if lnw is not None:
            W64 = W64 * lnw.astype(np.float64)[None, :]
        return W64.astype(np.float32)

    ids = np.asarray(I['input_ids'], np.int64)
    txt = I['embed'][ids]                                    # (B, T, D)
    txtT = txt.reshape(B * T, D).T.astype(np.float32)        # (D, B*T) order: b-major cols
    txt16 = _h(txtT).reshape(KT, 128, B * T).transpose(1, 0, 2)   # [128, KT, B*T]

    inv = 1.0 / (10000.0 ** (np.arange(0, HD, 2, dtype=np.float64) / HD))
    ang = np.arange(S, dtype=np.float64)[:, None] * inv[None, :]
    cosT = _h(np.ascontiguousarray(np.concatenate([np.cos(ang), np.cos(ang)], 1).T))
    sinT = _h(np.ascontiguousarray(np.concatenate([-np.sin(ang), np.sin(ang)], 1).T))

    # one shifted master mask: mbig[j, u] = 0 iff j <= u - 256; slice
    # [:, 256-128*jtl : 640-128*jtl] recovers the per-key-block causal mask
    jj = np.arange(128)[:, None]
    uu = np.arange(640)[None, :]
    mbig = np.where(jj <= uu - 256, 0.0, MASK_NEG).astype(np.float32)

    imgT = I['image_embeds'].reshape(B * NIMG, VH).T.astype(np.float32)   # (VH, B*NIMG)
    imp = np.zeros((VK, 128, B * NIMGP), np.float32)
    for k in range(VK):
        for b in range(B):
            imp[k, :, b * NIMGP:b * NIMGP + NIMG] = imgT[k * 128:(k + 1) * 128,
                                                         b * NIMG:(b + 1) * NIMG]
    img16 = _h(imp.transpose(1, 0, 2).reshape(128, VK * B * NIMGP))

    projT = I['proj_W'].astype(np.float32).T                 # (VH, D)
    pw = projT.reshape(VK, 128, KT, 128).transpose(2, 1, 0, 3)        # [m,p,k,f]
    pw = pw.reshape(KT // 4, 4, 128, VK, 128).transpose(0, 2, 1, 3, 4)
    projw16 = _h(pw.reshape(KT // 4, 128, 4 * VK * 128))

    projb_t = np.ascontiguousarray(I['proj_b'].astype(np.float32).reshape(KT, 128).T)
    lnf_t = np.ascontiguousarray(I['ln_f'].astype(np.float32).reshape(KT, 128).T)

    shared = dict(
        projb=projb_t, txt=np.ascontiguousarray(txt16), cos_t=cosT, sin_t=sinT,
        mbig=_bf(mbig),
        onesb=_h(np.ones((128, 1), np.float32)),
        onesrow=_h(np.ones((1, 128), np.float32)),
        onesrow32=np.ones((1, 128), np.float32),
        sel4=np.ascontiguousarray(
            np.where(np.arange(128)[:, None] % 32 == 0, 1.0, 0.0).astype(np.float32)),
        ident=_h(np.eye(128, dtype=np.float32)),
        lnf=lnf_t,
        img=img16, projw=projw16,
    )

    per_core = [dict(shared) for _ in range(NCORES)]

    for l in range(NL):
        Wq = fold(I['Wq'][l], I['Aq'][l], I['Bq'][l], I['ln1'][l])
        Wk = fold(I['Wk'][l], I['Ak'][l], I['Bk'][l], I['ln1'][l])
        Wv = fold(I['Wv'][l], I['Av'][l], I['Bv'][l], I['ln1'][l])
        Wo = fold(I['Wo'][l], I['Ao'][l], I['Bo'][l])
        Wg = fold(I['Wg'][l], I['Ag'][l], I['Bg'][l], I['ln2'][l])
        Wu = fold(I['Wu'][l], I['Au'][l], I['Bu'][l], I['ln2'][l])
        Wd = fold(I['Wd'][l], I['Ad'][l], I['Bd'][l])
        for r in range(NCORES):
            qs = Wq[r * DSH:(r + 1) * DSH]
            ks = Wk[r * HD:(r + 1) * HD]
            vs = Wv[r * HD:(r + 1) * HD]
            wqkvT = np.vstack([qs, ks, vs]).T                # (D, 768)
            arr = wqkvT.reshape(KT, 128, 6, 128).transpose(2, 1, 0, 3)   # [m,p,k,f]
            per_core[r][f"wqkv{l}"] = _h(arr.reshape(6, 128, KT * 128))

            woT = Wo[:, r * DSH:(r + 1) * DSH].T             # (512, D) rows=k-in, cols=d-out
            arr = woT.reshape(QH, 128, KT, 128).transpose(2, 1, 0, 3)    # [m,p,k,f]
            arr = arr.reshape(KT // 4, 4, 128, QH, 128).transpose(0, 2, 1, 3, 4)
            per_core[r][f"wo{l}"] = _h(arr.reshape(KT // 4, 128, 4 * QH * 128))

            gT = Wg[r * FSH:(r + 1) * FSH].T                 # (D, FSH)
            uT = Wu[r * FSH:(r + 1) * FSH].T
            ga = gT.reshape(KT, 128, FT, 128).transpose(2, 1, 0, 3).reshape(FT, 128, KT * 128)
            ua = uT.reshape(KT, 128, FT, 128).transpose(2, 1, 0, 3).reshape(FT, 128, KT * 128)
            per_core[r][f"wgu{l}"] = _h(np.stack([ga, ua], axis=1))

            wdT = Wd[:, r * FSH:(r + 1) * FSH].T             # (FSH, D)
            arr = wdT.reshape(FT, 128, KT, 128).transpose(2, 1, 0, 3)    # [m,p,j,f]
            arr = arr.reshape(KT // 2, 2, 128, FT, 128).transpose(0, 2, 1, 3, 4)
            per_core[r][f"wd{l}"] = _h(arr.reshape(KT // 2, 128, 2 * FT * 128))
    return per_core


def kernel(**inputs):
    global _PROGRAM
    from concourse.bass_utils import run_bass_kernel_spmd

    in_maps = _host_prep(inputs)
    if _PROGRAM is None:
        _PROGRAM = _build_program()
    res = None
    for attempt in range(3):
        try:
            res = run_bass_kernel_spmd(_PROGRAM, in_maps, list(range(NCORES)))
            break
        except Exception as e:
            if attempt == 2 or 'UNAVAILABLE' not in str(type(e).__name__) + str(e):
                raise
    out = np.asarray(res.results[0]["out"], np.float32)      # [128, KT, B*S]
    full = out.reshape(128, KT, B, S).transpose(2, 3, 1, 0)  # (B, S, KT, 128)
    return np.ascontiguousarray(full.reshape(B, S, D))
